# revision 15
# baseline (speedup 1.0000x reference)
"""Birman-Schwinger core: K[b] = diag(sqrt|V_b|) @ R_0 @ diag(sqrt|V_b|).

With g[b,u] = sqrt(|V[b,u]| + eps) / (1 + u) and d = u - v:

    K[b,u,v] = g[b,u] * g[b,v] * H(d)
    H(d) = -0.5*sign(d)*sin(2d) + 0.5j*sign(d)*cos(2d)

Angle-difference identities make every tile a sign-masked rank-2 outer
product (a_u = g_u sin 2u, b_u = g_u cos 2u, c_v = g_v cos 2v,
s_v = g_v sin 2v), which the TensorEngine produces as K=6 bf16 matmuls
(hi/lo bf16 splits give ~fp32 accuracy). The kernel is HBM-store-bound
(output leaves as interleaved fp16, host upcasts to complex64), and the
f32->fp16 PSUM drain (1 elem/cycle on ScalarE/DVE) is the engine
bottleneck, so work is split two ways:

- Diagonal-band row blocks (program slots 0..15) are produced entirely on
  the Vector engine from an on-chip Toeplitz table T[p, tau] =
  H(1920 + p - tau) (fp16, built once by matmul + drain, diagonal sign
  flip and zeros baked in): one 4x tensor_scalar forms g_u*g_v and one
  2x tensor_tensor multiplies the sliding table window - no PSUM drain.
- Off-diagonal blocks (slots 16..31, sign uniform per core) go through
  the TensorEngine -> PSUM -> ScalarE fp16 drain.

Sharding: 8 cores; core c handles batch b = c // 2 and column half
h = c % 2 (all 4096 rows x 2048 complex columns). Row blocks are
processed in the order (s + 16h) % 32 so banded blocks occupy slots
0..15 on every core - the instruction stream is identical across cores,
only the factor data differs; the host un-permutes blocks on assembly.
"""

import numpy as np

B = 4
N = 4096
NCORES = 8
P = 128                  # SBUF partitions
NSLOT = N // P           # 32 row blocks per core
NLOC = N // 2            # complex columns per core (column half)
EPS = 1e-10
FW = 2 * NLOC            # f16 columns per block row (4096)
PS = 2048                # f32 columns per PSUM drain chunk (4 banks)
TC = 3968                # table width in complex columns
TBASE = 1920             # table diagonal offset: T[p, tau] = H(1920 + p - tau)

_PROGRAM_CACHE = {}


def _build_program():
    import concourse.bacc as bacc
    import concourse.mybir as mybir
    from concourse.tile import TileContext

    nc = bacc.Bacc("TRN2", target_bir_lowering=False, debug=False)
    lhs_t = nc.dram_tensor(
        "t_lhs_t", [6, 2 * P], mybir.dt.bfloat16, kind="ExternalInput"
    ).ap()
    rhs_t = nc.dram_tensor(
        "t_rhs_t", [6, 2 * TC], mybir.dt.bfloat16, kind="ExternalInput"
    ).ap()
    lhs_m = nc.dram_tensor(
        "t_lhs_m", [6, 16 * P], mybir.dt.bfloat16, kind="ExternalInput"
    ).ap()
    rhs_m = nc.dram_tensor(
        "t_rhs_m", [6, FW], mybir.dt.bfloat16, kind="ExternalInput"
    ).ap()
    rhs_g = nc.dram_tensor(
        "t_rhs_g", [3, FW], mybir.dt.bfloat16, kind="ExternalInput"
    ).ap()
    ones = nc.dram_tensor(
        "t_ones", [3, P], mybir.dt.bfloat16, kind="ExternalInput"
    ).ap()
    gu = nc.dram_tensor("t_gu", [P, 16], mybir.dt.float32, kind="ExternalInput").ap()
    mask = nc.dram_tensor(
        "t_mask", [P, 2 * P], mybir.dt.float16, kind="ExternalInput"
    ).ap()
    out = nc.dram_tensor(
        "t_out", [N, FW], mybir.dt.float16, kind="ExternalOutput"
    ).ap()
    mult = mybir.AluOpType.mult

    with TileContext(nc) as tc:
        with tc.tile_pool(name="const", bufs=1) as cpool:
            tab_sb = cpool.tile([P, 2 * TC], mybir.dt.float16)
            gvb_sb = cpool.tile([P, FW], mybir.dt.float16)
            lhs_t_sb = cpool.tile([6, 2 * P], mybir.dt.bfloat16)
            rhs_t_sb = cpool.tile([6, 2 * TC], mybir.dt.bfloat16)
            lhs_m_sb = cpool.tile([6, 16 * P], mybir.dt.bfloat16)
            rhs_m_sb = cpool.tile([6, FW], mybir.dt.bfloat16)
            rhs_g_sb = cpool.tile([3, FW], mybir.dt.bfloat16)
            ones_sb = cpool.tile([3, P], mybir.dt.bfloat16)
            gu_sb = cpool.tile([P, 16], mybir.dt.float32)
            mask_sb = cpool.tile([P, 2 * P], mybir.dt.float16)
            nc.sync.dma_start(out=ones_sb[:, :], in_=ones[:, :])
            nc.sync.dma_start(out=rhs_g_sb[:, :], in_=rhs_g[:, :])
            nc.sync.dma_start(out=lhs_t_sb[:, :], in_=lhs_t[:, :])
            nc.sync.dma_start(out=rhs_t_sb[:, :], in_=rhs_t[:, :])
            nc.sync.dma_start(out=lhs_m_sb[:, :], in_=lhs_m[:, :])
            nc.sync.dma_start(out=rhs_m_sb[:, :], in_=rhs_m[:, :])
            nc.sync.dma_start(out=gu_sb[:, :], in_=gu[:, :])
            nc.sync.dma_start(out=mask_sb[:, :], in_=mask[:, :])

            with (
                tc.tile_pool(name="work", bufs=6) as wpool,
                tc.tile_pool(name="gvs", bufs=2) as gpool,
                tc.tile_pool(name="psum", bufs=2, space="PSUM") as ppool,
            ):
                def build_chunk(dst, dst_lo, width, mms):
                    """matmul chunk list into one PSUM tile, drain on DVE."""
                    pt = ppool.tile([P, PS], mybir.dt.float32, name="pt")
                    for o, nw, w_ap, r_ap in mms:
                        nc.tensor.matmul(
                            out=pt[:, o : o + nw],
                            lhsT=w_ap,
                            rhs=r_ap,
                            start=True,
                            stop=True,
                        )
                    nc.vector.tensor_copy(
                        out=dst[:, dst_lo : dst_lo + width], in_=pt[:, 0:width]
                    )

                # --- gvb: broadcast g_v to all partitions (fp16), via
                # ones^T @ (3-way bf16 split of g) on the TensorEngine.
                for q0 in range(0, FW, PS):
                    mms = [
                        (o, 512, ones_sb[:, :], rhs_g_sb[:, q0 + o : q0 + o + 512])
                        for o in range(0, PS, 512)
                    ]
                    build_chunk(gvb_sb, q0, PS, mms)

                # --- H table chunks 0..1 (f16 cols [0, 4096)), then the
                # diagonal band sign fix at f16 cols [3840, 4096).
                def tab_chunk(k):
                    mms = []
                    for o in range(0, PS, 512):
                        j0 = 2048 * k + o  # f16 col in table
                        if j0 >= 2 * TC:
                            break
                        tau0 = j0 // 2
                        nw = min(512, 2 * TC - j0)
                        v = 0 if tau0 < 2048 else 1  # band cols use +0.5 too
                        mms.append(
                            (
                                o,
                                nw,
                                lhs_t_sb[:, v * P : v * P + P],
                                rhs_t_sb[:, j0 : j0 + nw],
                            )
                        )
                    width = min(PS, 2 * TC - 2048 * k)
                    build_chunk(tab_sb, 2048 * k, width, mms)

                tab_chunk(0)
                tab_chunk(1)
                nc.vector.tensor_tensor(
                    out=tab_sb[:, 2 * TBASE : 2 * TBASE + 2 * P],
                    in0=tab_sb[:, 2 * TBASE : 2 * TBASE + 2 * P],
                    in1=mask_sb[:, :],
                    op=mult,
                )

                def m_block(s):
                    # off-diagonal slot: PE matmuls, ScalarE drains
                    w = wpool.tile([P, FW], mybir.dt.float16, name="w")
                    wv = lhs_m_sb[:, (s - 16) * P : (s - 15) * P]
                    for half in range(FW // PS):
                        pt = ppool.tile([P, PS], mybir.dt.float32, name="pt")
                        c_lo = PS * half
                        for o in range(0, PS, 512):
                            nc.tensor.matmul(
                                out=pt[:, o : o + 512],
                                lhsT=wv,
                                rhs=rhs_m_sb[:, c_lo + o : c_lo + o + 512],
                                start=True,
                                stop=True,
                            )
                        nc.scalar.copy(out=w[:, c_lo : c_lo + PS], in_=pt[:, :])
                    nc.sync.dma_start(out=out[s * P : (s + 1) * P, :], in_=w[:, :])

                def s_block(s):
                    # banded slot: all-DVE from the table window
                    w = wpool.tile([P, FW], mybir.dt.float16, name="w")
                    gvs = gpool.tile([P, FW], mybir.dt.float16, name="gvs")
                    nc.vector.tensor_scalar(
                        out=gvs[:, :],
                        in0=gvb_sb[:, :],
                        scalar1=gu_sb[:, s : s + 1],
                        scalar2=None,
                        op0=mult,
                    )
                    w0 = 2 * TBASE - 256 * s
                    nc.vector.tensor_tensor(
                        out=w[:, :],
                        in0=tab_sb[:, w0 : w0 + FW],
                        in1=gvs[:, :],
                        op=mult,
                    )
                    nc.sync.dma_start(out=out[s * P : (s + 1) * P, :], in_=w[:, :])

                m_block(16)
                tab_chunk(2)
                m_block(17)
                tab_chunk(3)
                m_block(18)
                m_block(19)
                for i in range(12):
                    s_block(15 - i)
                    m_block(20 + i)
                for s in range(3, -1, -1):
                    s_block(s)
    nc.compile()
    return nc


def _get_program():
    if "nc" not in _PROGRAM_CACHE:
        _PROGRAM_CACHE["nc"] = _build_program()
    return _PROGRAM_CACHE["nc"]


def _host_tables(V):
    """Per-core input arrays (bf16 hi/lo-split trig factor tables)."""
    import ml_dtypes

    bf16 = ml_dtypes.bfloat16

    def split2(x):
        hi = x.astype(bf16)
        lo = (x - hi.astype(np.float64)).astype(bf16)
        return hi, lo

    def rank2_rhs(cos_t, sin_t):
        """rhs rows pairing with lhs rows [A0,A0,A1,B0,B0,B1]."""
        c0, c1 = split2(cos_t)
        s0, s1 = split2(sin_t)
        m = len(cos_t)
        r = np.empty((6, 2 * m), dtype=bf16)
        r[0, 0::2] = -c0
        r[0, 1::2] = s0
        r[1, 0::2] = -c1
        r[1, 1::2] = s1
        r[2] = r[0]
        r[3, 0::2] = s0
        r[3, 1::2] = c0
        r[4, 0::2] = s1
        r[4, 1::2] = c1
        r[5] = r[3]
        return r

    def rank2_lhs(a, bb):
        """lhs rows [A0, A0, A1, B0, B0, B1] for row factors a, b (f64)."""
        A0, A1 = split2(a)
        B0, B1 = split2(bb)
        return np.stack([A0, A0, A1, B0, B0, B1])

    pos = np.arange(N, dtype=np.float64)
    g = np.sqrt(np.abs(V).astype(np.float64) + EPS) / (1.0 + pos)  # (B, N) f64
    sin2 = np.sin(2.0 * pos)
    cos2 = np.cos(2.0 * pos)

    # H table build factors (g-independent, same for every core)
    p_ = np.arange(P, dtype=np.float64)
    alpha = 2.0 * (TBASE + p_)
    lhs_t = np.empty((6, 2 * P), dtype=bf16)
    lhs_t[:, 0:P] = rank2_lhs(0.5 * np.sin(alpha), 0.5 * np.cos(alpha))
    lhs_t[:, P : 2 * P] = rank2_lhs(-0.5 * np.sin(alpha), -0.5 * np.cos(alpha))
    tau = np.arange(TC, dtype=np.float64)
    rhs_t = rank2_rhs(np.cos(2.0 * tau), np.sin(2.0 * tau))

    pq = np.arange(P, dtype=np.int64)
    sgn = np.sign(pq[:, None] - pq[None, :]).astype(np.float16)
    mask = np.empty((P, 2 * P), dtype=np.float16)
    mask[:, 0::2] = sgn
    mask[:, 1::2] = sgn

    in_maps = []
    for core in range(NCORES):
        b, h = divmod(core, 2)
        vloc = np.arange(NLOC, dtype=np.int64) + NLOC * h
        gloc = g[b, vloc]

        # M-mode column factors (g-weighted)
        rhs_m = rank2_rhs(gloc * cos2[vloc], gloc * sin2[vloc])

        # M-mode row factors for slots 16..31
        sigma = 1.0 if h == 0 else -1.0
        lhs_m = np.empty((6, 16 * P), dtype=bf16)
        for s in range(16, NSLOT):
            j = (s + 16 * h) % NSLOT
            u = 128 * j + pq
            lhs_m[:, (s - 16) * P : (s - 15) * P] = rank2_lhs(
                0.5 * sigma * g[b, u] * sin2[u], 0.5 * sigma * g[b, u] * cos2[u]
            )

        # gvb build: 3-way bf16 split of interleave-duplicated g
        grow = np.empty(FW, dtype=np.float64)
        grow[0::2] = gloc
        grow[1::2] = gloc
        rhs_g = np.empty((3, FW), dtype=bf16)
        rhs_g[0] = grow.astype(bf16)
        r1 = grow - rhs_g[0].astype(np.float64)
        rhs_g[1] = r1.astype(bf16)
        r2 = r1 - rhs_g[1].astype(np.float64)
        rhs_g[2] = r2.astype(bf16)

        # per-partition g_u scalars for banded slots 0..15
        gu_t = np.empty((P, 16), dtype=np.float32)
        for s in range(16):
            j = (s + 16 * h) % NSLOT
            gu_t[:, s] = g[b, 128 * j + pq]

        in_maps.append(
            {
                "t_lhs_t": lhs_t,
                "t_rhs_t": rhs_t,
                "t_lhs_m": lhs_m,
                "t_rhs_m": rhs_m,
                "t_rhs_g": rhs_g,
                "t_ones": np.ones((3, P), dtype=bf16),
                "t_gu": gu_t,
                "t_mask": mask,
            }
        )
    return in_maps


def _run(in_maps, trace=False, **kwargs):
    from concourse import bass_utils

    nc = _get_program()
    return bass_utils.run_bass_kernel_spmd(
        nc, in_maps, core_ids=list(range(NCORES)), trace=trace, **kwargs
    )


def kernel(V):
    V = np.asarray(V, dtype=np.float32)
    assert V.shape == (B, N), V.shape
    in_maps = _host_tables(V)
    res = _run(in_maps, trace=False)
    out = np.empty((B, N, N), dtype=np.complex64)
    slot = np.arange(NSLOT)
    for core in range(NCORES):
        b, h = divmod(core, 2)
        plane = np.asarray(res.results[core]["t_out"], dtype=np.float32).view(
            np.complex64
        )  # (4096, 2048), rows in slot order
        j = (slot + 16 * h) % NSLOT  # slot -> global row block
        dst = out[b, :, NLOC * h : NLOC * (h + 1)].reshape(NSLOT, P, NLOC)
        dst[j] = plane.reshape(NSLOT, P, NLOC)
    return out
